# revision 46
# baseline (speedup 1.0000x reference)
"""Trainium2 Bass kernel for leave-one-out Nadaraya-Watson regression
(nn_Net_72877005078649) — fast-Gauss-transform (Taylor moment) algorithm.

Math:
  Xw = mlp(train_X) [N,10], Zw = mlp(x) [B,10]  (mlp = W2 @ relu(W1 @ .))
  K[b,n,o] = exp(-0.5*((Xw[n,o]-Zw[b,o])/h)^2), K[b,b,:] = 0
  out[b,o] = sum_n K*Y[n,o] / sum_n K

Key reformulation (x' = Xw/h, z' = Zw/h):
  K = e^{-x'^2/2} * e^{x' z'} * e^{-z'^2/2}; the last factor is constant
  over n and cancels in the num/den ratio.  Expanding e^{x'z'} in a
  KT-term Taylor series collapses the O(B*N*O) kernel sum to per-channel
  moments:
    num[b,o] = sum_k z'^k/k! * M_k,o,   M_k,o = sum_n Y[n,o] e^{-x'^2/2} x'^k
  (den likewise with Y:=1).  max |x' z'| ~ 4.7 on this data; KT=12 terms
  give rel err ~3e-4 end-to-end (measured on HW) vs the 2e-2 gate —
  per-element truncation error is diluted by the 4096-term positive n-sum.

Device pipeline per core (B sharded 8 ways -> 512 queries/core; N, Y, W
replicated; no collectives):
  ALL inputs arrive in device layout: host pre-transposes, casts to bf16
  and STACKS each 512-col quad as [128, 256] (top rows = cols 0:256,
  bottom = cols 256:512) paired with dual weight variants [W1t;0]|[0;W1t]
  — K=128 matmuls both run at full rate and count toward the PE HAM
  activity monitor (K<128 and fp32 matmuls never unthrottle the 1.2->2.4
  GHz clock gate) without doubling the DMA bytes; every DMA is
  contiguous (a [128,(t,o)] gather of Y generated ~4600 tiny descriptors
  and stalled the input stream by 7us in an earlier rev); tXT arrives in
  4 staggered pieces, one SBUF tile each (a shared tile made every MM1
  conservatively wait on ALL writers)
  5 bf16 K=128 dummy matmuls warm HAM while the inputs stream in, and
  keep-alive matmuls (one set gated on cd) bridge the DVE-chain gap so
  the moment matmuls stay at 2.4 GHz
  MM1 (bf16, dual w1 variants) + relu (alternating ACT/DVE) -> H bf16;
  MM2 per 128-row tile (lhsT=H tile) -> X' = Xw/h in one PSUM bank
  d = exp(-x'^2/2) (ACT), c = Y*d (GPSIMD) -> cd [128,(32,2,10)] f32r
  power table V[128,(36,10,12)] f32r built by a square/multiply
  ping-pong: even powers x^2m = Square(x^m) on ACT, odd powers on DVE —
  the serial chain spans two engines instead of 11 serial DVE multiplies
  (the Tile scheduler interleaves independent DVE ops, so splitting the
  chain in halves never overlapped anything)
  moments: 32 accumulated matmuls lhsT=cd[128,20] rhs=V[128,120] ->
    PSUM [20,(o,k)]; only the o==o' diagonal blocks are used
  diag-select + 1/k!-scale via host-built masks (2 DVE mults), then a
    ones-weighted matmul broadcasts M to all 128 partitions
  eval num/den = one DVE mult over all 4 query tiles (M2 stride-0
    broadcast) + one tensor_reduce over k each, exact leave-one-out
    diagonal subtraction from the td/yd projections, divide; output
    leaves in device layout [128,(4,10)], host un-permutes.
"""

import numpy as np

N = 4096
D = 64
HID = 128
O = 10
NCORES = 8
BQ = N // NCORES          # queries per core (512)
KT = 12                   # Taylor terms
NT_SRC = N // 128         # 32 source tiles
NTILES = NT_SRC + 8       # + 4 query tiles + 4 diag tiles
NPOW = NT_SRC + 4         # chunks carrying power tables (src + query)
NQUAD = NTILES // 4       # 10 quads of 512 rows

_cache = {}


def _build(h: float):
    import concourse.bass as bass
    import concourse.bacc as bacc
    import concourse.tile as tile
    from concourse import mybir

    f32 = mybir.dt.float32
    f32r = mybir.dt.float32r
    bf16 = mybir.dt.bfloat16
    AF = mybir.ActivationFunctionType
    ALU = mybir.AluOpType

    nc = bacc.Bacc("TRN2", target_bir_lowering=False, debug=False, num_devices=1)
    # each 512-col quad is host-stacked to [128, 256] (top=cols 0:256,
    # bottom=cols 256:512) so K=128 without doubling the DMA bytes;
    # w1T carries both weight variants [W1t;0] | [0;W1t]
    xqT = nc.dram_tensor("xqT", [HID, BQ // 2], bf16, kind="ExternalInput").ap()
    tXT = nc.dram_tensor("tXT", [HID, N // 2], bf16, kind="ExternalInput").ap()
    tdT = nc.dram_tensor("tdT", [HID, BQ // 2], bf16, kind="ExternalInput").ap()
    w1T = nc.dram_tensor("w1T", [HID, 2 * HID], bf16, kind="ExternalInput").ap()
    w2T = nc.dram_tensor("w2T", [HID, O], bf16, kind="ExternalInput").ap()
    Yt = nc.dram_tensor("Y", [128, NT_SRC * O], bf16, kind="ExternalInput").ap()
    yd = nc.dram_tensor("yd", [128, 4 * O], bf16, kind="ExternalInput").ap()
    EJ = nc.dram_tensor("EJ", [20, 2 * O * KT], f32, kind="ExternalInput").ap()
    out = nc.dram_tensor("out", [128, 4 * O], f32, kind="ExternalOutput").ap()

    with tile.TileContext(nc) as tc:
        with (
            tc.tile_pool(name="singles", bufs=1) as S,
            tc.tile_pool(name="work", bufs=3) as W,
            tc.tile_pool(name="psW", bufs=1, space="PSUM") as PSW,
            tc.tile_pool(name="psH", bufs=3, space="PSUM") as PSH,
            tc.tile_pool(name="psX", bufs=1, space="PSUM") as PSX,
            tc.tile_pool(name="psM", bufs=1, space="PSUM") as PSM,
        ):
            # ---------------- constants ----------------
            warm = S.tile([1, 16], f32)
            nc.vector.memset(warm, 0.0)
            nc.scalar.activation(out=warm, in_=warm, func=AF.Exp)
            ones128 = S.tile([128, 512], bf16)
            nc.vector.memset(ones128, 1.0)
            ones32f = S.tile([20, 128], f32)
            nc.vector.memset(ones32f, 1.0)
            onesW = S.tile([20, 128], f32r)
            nc.vector.tensor_copy(onesW, ones32f)
            vones = S.tile([128, NPOW * O], f32)
            nc.vector.memset(vones, 1.0)

            # PE HAM warm-up: K=128 bf16 matmuls while inputs stream in
            wps = PSW.tile([128, 512], f32, tag="warm", name="wps")
            for i in range(5):
                nc.tensor.matmul(wps, lhsT=ones128[:, 0:128], rhs=ones128,
                                 start=True, stop=True)

            # -------- input DMAs: weights first, tXT staggered ----------
            # one SBUF tile per DMA: a shared tile would make every MM1
            # conservatively wait on ALL xT writers
            w1sb = S.tile([HID, 2 * HID], bf16)
            nc.sync.dma_start(out=w1sb, in_=w1T)
            w2sb = S.tile([HID, O], bf16)
            nc.sync.dma_start(out=w2sb, in_=w2T)
            cuts = [0, 256, 768, 1280, 2048]     # stacked coords
            xTp = []
            for i in range(4):
                pc = cuts[i + 1] - cuts[i]
                t = S.tile([HID, pc], bf16, name=f"xTp{i}")
                eng = nc.sync if i % 2 == 0 else nc.gpsimd
                eng.dma_start(out=t, in_=tXT[:, cuts[i]:cuts[i + 1]])
                xTp.append(t)
            xTq = S.tile([HID, BQ // 2], bf16, name="xTq")
            nc.sync.dma_start(out=xTq, in_=xqT)
            xTd = S.tile([HID, BQ // 2], bf16, name="xTd")
            nc.gpsimd.dma_start(out=xTd, in_=tdT)
            # quad q -> (piece tile, col offset within piece)
            quad_src = []
            for q in range(8):
                base = q * 256
                i = next(i for i in range(4) if cuts[i] <= base < cuts[i + 1])
                quad_src.append((xTp[i], base - cuts[i]))
            quad_src += [(xTq, 0), (xTd, 0)]
            Ej = S.tile([20, 2 * O * KT], f32)
            nc.scalar.dma_start(out=Ej, in_=EJ)
            Ytab = S.tile([128, NT_SRC * O], bf16)
            nc.scalar.dma_start(out=Ytab, in_=Yt)
            ydT = S.tile([128, 4 * O], bf16)
            nc.scalar.dma_start(out=ydT, in_=yd)

            # ---------------- MM1 + relu -> H ----------------
            H = S.tile([128, NTILES * 128], bf16)
            for q in range(NQUAD):
                src, off = quad_src[q]
                hp = PSH.tile([128, 512], f32, tag="H", name="hps")
                for hh in range(2):
                    nc.tensor.matmul(hp[:, hh * 256:(hh + 1) * 256],
                                     lhsT=w1sb[:, hh * HID:(hh + 1) * HID],
                                     rhs=src[:, off:off + 256],
                                     start=True, stop=True)
                dst = H[:, q * 512:(q + 1) * 512]
                if q % 2 == 0:
                    nc.scalar.activation(out=dst, in_=hp, func=AF.Relu)
                else:
                    nc.vector.tensor_scalar_max(dst, hp, 0.0)

            # ---------------- MM2 -> X' ----------------
            xps = PSX.tile([128, NTILES * O], f32, tag="xp", name="xps")
            for t in range(NTILES):
                nc.tensor.matmul(
                    xps[:, t * O:(t + 1) * O],
                    lhsT=H[:, t * 128:(t + 1) * 128], rhs=w2sb,
                    start=True, stop=True)
            Xp = S.tile([128, NTILES * O], f32)
            nc.scalar.copy(Xp, xps)

            QC = NT_SRC * O          # col offset of query block (320)
            DC = (NT_SRC + 4) * O    # col offset of diag block (360)

            # ---------------- d, c -> cd table ----------------
            sq = S.tile([128, NTILES * O], f32)
            nc.scalar.square(sq, Xp)
            cd = S.tile([128, NT_SRC * 2 * O], f32r)
            cd4 = cd.rearrange("p (c j o) -> p c j o", j=2, o=O)
            nc.scalar.activation(
                out=cd4[:, :, 1, :],
                in_=sq.rearrange("p (c o) -> p c o", o=O)[:, 0:NT_SRC, :],
                func=AF.Exp, scale=-0.5)
            nc.gpsimd.tensor_mul(cd4[:, :, 0, :],
                                 Ytab.rearrange("p (c o) -> p c o", o=O),
                                 cd4[:, :, 1, :])
            # PE keep-alive: the DVE power chain leaves the PE idle >3.4us,
            # which re-throttles HAM; a few K=128 matmuls (one set gated on
            # cd so it fires mid-gap) keep the clock at 2.4 GHz
            onesR = S.tile([128, 128], f32r)
            nc.vector.tensor_copy(onesR, ones128[:, 0:128])
            for i in range(3):
                nc.tensor.matmul(wps[:, 0:128], lhsT=onesR, rhs=onesR,
                                 start=True, stop=True)
            for i in range(3):
                nc.tensor.matmul(wps[:, 0:128], lhsT=onesR, rhs=cd[:, 0:128],
                                 start=True, stop=True)

            # -------- power table: square/multiply ping-pong --------
            # x^2m = Square(x^m) on ACT, x^(2m+1) = x^2m * x on DVE;
            # the serial chain spans two engines instead of one
            V = S.tile([128, NPOW * O * KT], f32r)
            V4 = V.rearrange("p (c o k) -> p c o k", o=O, k=KT)
            Xs4 = Xp.rearrange("p (c o) -> p c o", o=O)[:, 0:NPOW, :]
            nc.vector.tensor_copy(V4[:, :, :, 0],
                                  vones.rearrange("p (c o) -> p c o", o=O))
            nc.scalar.copy(V4[:, :, :, 1], Xs4)
            for k in range(2, KT):
                if k % 2 == 0:
                    nc.scalar.square(V4[:, :, :, k], V4[:, :, :, k // 2])
                else:
                    nc.vector.tensor_mul(V4[:, :, :, k], V4[:, :, :, k - 1],
                                         Xs4)

            # ---------------- moment matmuls ----------------
            mps = PSM.tile([20, O * KT], f32, tag="M", name="mps")
            for c in range(NT_SRC):
                nc.tensor.matmul(
                    mps, lhsT=cd[:, c * 2 * O:(c + 1) * 2 * O],
                    rhs=V[:, c * O * KT:(c + 1) * O * KT],
                    start=(c == 0), stop=(c == NT_SRC - 1))

            # select diag blocks M[j*10+o, (o,k)] (1/k! in the mask) and
            # broadcast to 128 partitions via a ones-weighted matmul
            masked = S.tile([20, 2 * O * KT], f32r)
            nc.vector.tensor_mul(masked[:, 0:O * KT], mps, Ej[:, 0:O * KT])
            nc.vector.tensor_mul(masked[:, O * KT:], mps, Ej[:, O * KT:])
            m2ps = PSX.tile([128, 2 * O * KT], f32, tag="m2", name="m2ps")
            nc.tensor.matmul(m2ps, lhsT=onesW, rhs=masked, start=True, stop=True)
            M2 = S.tile([128, 2 * O * KT], f32)
            nc.scalar.copy(M2, m2ps)

            # ---------------- eval ----------------
            num = S.tile([128, 4 * O], f32)
            den = S.tile([128, 4 * O], f32)
            M2P = M2.ap[0][0]
            UQ = V[:, NT_SRC * O * KT:NPOW * O * KT]   # query powers
            for j, acc in ((0, num), (1, den)):
                m2b = bass.AP(tensor=M2.tensor, offset=M2.offset + j * O * KT,
                              ap=[[M2P, 128], [0, 4], [1, O * KT]])
                p1 = W.tile([128, 4 * O * KT], f32, tag="p1")
                nc.vector.tensor_mul(
                    p1.rearrange("p (qc f) -> p qc f", f=O * KT),
                    UQ.rearrange("p (qc f) -> p qc f", f=O * KT), m2b)
                nc.vector.tensor_reduce(
                    acc, p1.rearrange("p (qc o k) -> p qc o k", o=O, k=KT),
                    axis=mybir.AxisListType.X, op=ALU.add)

            # ---------------- diagonal correction ----------------
            t1 = S.tile([128, 4 * O], f32)
            nc.vector.tensor_mul(t1, Xp[:, DC:DC + 4 * O], Xp[:, QC:QC + 4 * O])
            nc.vector.scalar_tensor_tensor(
                out=t1, in0=sq[:, DC:DC + 4 * O], scalar=-0.5, in1=t1,
                op0=ALU.mult, op1=ALU.add)
            kd = S.tile([128, 4 * O], f32)
            nc.scalar.activation(out=kd, in_=t1, func=AF.Exp)
            nc.vector.tensor_mul(t1, kd, ydT)
            nc.vector.tensor_sub(num, num, t1)
            nc.vector.tensor_sub(den, den, kd)
            rec = S.tile([128, 4 * O], f32)
            nc.vector.reciprocal(rec, den)
            nc.vector.tensor_mul(num, num, rec)

            nc.sync.dma_start(out=out, in_=num)

    nc.compile()
    return nc


def _ej_const():
    """[20, (j,o,k)] mask: row j*10+o keeps block (j, o, :) with value 1/k!."""
    ej = np.zeros((20, 2 * O * KT), np.float32)
    fact = np.cumprod(np.concatenate([[1.0], np.arange(1, KT)])).astype(np.float64)
    for j in range(2):
        for o in range(O):
            ej[j * O + o, (j * O + o) * KT:(j * O + o + 1) * KT] = 1.0 / fact
    return ej


def _stackT(a):
    """[n, 64] f32 -> [128, n/2] bf16: per 512-col quad of a.T, top half
    rows hold cols 0:256, bottom half rows hold cols 256:512 (K=128
    matmuls with the dual zero-padded weight variants, no extra bytes)."""
    import ml_dtypes
    T = a.T.astype(np.float32)                    # [64, n]
    nq = T.shape[1] // 512
    Q = T.reshape(D, nq, 2, 256)
    out = np.concatenate([Q[:, :, 0, :], Q[:, :, 1, :]], axis=0)  # [128,nq,256]
    return np.ascontiguousarray(out.reshape(HID, nq * 256)).astype(ml_dtypes.bfloat16)


def make_in_maps(x, train_X, Y, W1, W2, h):
    import ml_dtypes
    bf = ml_dtypes.bfloat16
    x = np.ascontiguousarray(x, dtype=np.float32)
    train_X = np.ascontiguousarray(train_X, dtype=np.float32)
    Y = np.asarray(Y, np.float32)
    # device layout [128, (t, o)]: partition p holds rows t*128+p
    Ydev = np.ascontiguousarray(
        Y.reshape(NT_SRC, 128, O).transpose(1, 0, 2).reshape(128, NT_SRC * O)
    ).astype(bf)
    tXTp = _stackT(train_X)
    w1f = np.asarray(W1, np.float32).T             # [64, 128]
    zer = np.zeros_like(w1f)
    w1t = np.ascontiguousarray(np.concatenate(
        [np.concatenate([w1f, zer], 0), np.concatenate([zer, w1f], 0)],
        axis=1)).astype(bf)                        # [128, 256] = [w1a | w1b]
    w2t = np.ascontiguousarray(
        (np.asarray(W2, np.float32) / float(h)).T).astype(bf)
    ej = _ej_const()
    in_maps = []
    for c in range(NCORES):
        sl = slice(c * BQ, (c + 1) * BQ)
        yds = np.ascontiguousarray(
            Y[sl].reshape(4, 128, O).transpose(1, 0, 2).reshape(128, 4 * O)
        ).astype(bf)
        in_maps.append({
            "xqT": _stackT(x[sl]),
            "tXT": tXTp,
            "tdT": _stackT(train_X[sl]),
            "w1T": w1t, "w2T": w2t,
            "Y": Ydev, "yd": yds, "EJ": ej,
        })
    return in_maps


def kernel(x, train_X, Y, W1, W2, h):
    import concourse.bass_utils as bass_utils

    hval = float(h)
    key = ("fgt10", hval)
    if key not in _cache:
        _cache[key] = _build(hval)
    nc = _cache[key]

    in_maps = make_in_maps(x, train_X, Y, W1, W2, hval)
    res = bass_utils.run_bass_kernel_spmd(nc, in_maps, core_ids=list(range(NCORES)))
    outs = []
    for c in range(NCORES):
        o = res.results[c]["out"]                      # [128, (qc, o)]
        outs.append(o.reshape(128, 4, O).transpose(1, 0, 2).reshape(BQ, O))
    return np.concatenate(outs, axis=0)


# revision 47
# speedup vs baseline: 1.0430x; 1.0430x over previous
"""Trainium2 Bass kernel for leave-one-out Nadaraya-Watson regression
(nn_Net_72877005078649) — fast-Gauss-transform (Taylor moment) algorithm.

Math:
  Xw = mlp(train_X) [N,10], Zw = mlp(x) [B,10]  (mlp = W2 @ relu(W1 @ .))
  K[b,n,o] = exp(-0.5*((Xw[n,o]-Zw[b,o])/h)^2), K[b,b,:] = 0
  out[b,o] = sum_n K*Y[n,o] / sum_n K

Key reformulation (x' = Xw/h, z' = Zw/h):
  K = e^{-x'^2/2} * e^{x' z'} * e^{-z'^2/2}; the last factor is constant
  over n and cancels in the num/den ratio.  Expanding e^{x'z'} in a
  KT-term Taylor series collapses the O(B*N*O) kernel sum to per-channel
  moments:
    num[b,o] = sum_k z'^k/k! * M_k,o,   M_k,o = sum_n Y[n,o] e^{-x'^2/2} x'^k
  (den likewise with Y:=1).  max |x' z'| ~ 4.7 on this data; KT=12 terms
  give rel err ~3e-4 end-to-end (measured on HW) vs the 2e-2 gate —
  per-element truncation error is diluted by the 4096-term positive n-sum.

Device pipeline per core (B sharded 8 ways -> 512 queries/core; N, Y, W
replicated; no collectives):
  ALL inputs arrive in device layout: host pre-transposes, casts to bf16
  and STACKS each 512-col quad as [128, 256] (top rows = cols 0:256,
  bottom = cols 256:512) paired with dual weight variants [W1t;0]|[0;W1t]
  — K=128 matmuls both run at full rate and count toward the PE HAM
  activity monitor (K<128 and fp32 matmuls never unthrottle the 1.2->2.4
  GHz clock gate) without doubling the DMA bytes; every DMA is
  contiguous (a [128,(t,o)] gather of Y generated ~4600 tiny descriptors
  and stalled the input stream by 7us in an earlier rev); tXT arrives in
  4 staggered pieces, one SBUF tile each (a shared tile made every MM1
  conservatively wait on ALL writers)
  5 bf16 K=128 dummy matmuls warm HAM while the inputs stream in, and
  keep-alive matmuls (one set gated on cd) bridge the DVE-chain gap so
  the moment matmuls stay at 2.4 GHz
  MM1 (bf16, dual w1 variants) + relu (alternating ACT/DVE) -> H bf16;
  MM2 per 128-row tile (lhsT=H tile) -> X' = Xw/h in one PSUM bank
  d = exp(-x'^2/2) (ACT), c = Y*d (GPSIMD) -> cd [128,(32,2,10)] f32r
  power table V[128,(36,10,12)] f32r built by a square/multiply
  ping-pong: even powers x^2m = Square(x^m) on ACT, odd powers on DVE —
  the serial chain spans two engines instead of 11 serial DVE multiplies
  (the Tile scheduler interleaves independent DVE ops, so splitting the
  chain in halves never overlapped anything)
  moments: 32 accumulated matmuls lhsT=cd[128,20] rhs=V[128,120] ->
    PSUM [20,(o,k)]; only the o==o' diagonal blocks are used
  diag-select + 1/k!-scale via host-built masks (2 DVE mults), then a
    ones-weighted matmul broadcasts M to all 128 partitions
  eval num/den = one DVE mult over all 4 query tiles (M2 stride-0
    broadcast) + one tensor_reduce over k each, exact leave-one-out
    diagonal subtraction from the td/yd projections, divide; output
    leaves in device layout [128,(4,10)], host un-permutes.
"""

import numpy as np

N = 4096
D = 64
HID = 128
O = 10
NCORES = 8
BQ = N // NCORES          # queries per core (512)
KT = 10                   # Taylor terms
NT_SRC = N // 128         # 32 source tiles
NTILES = NT_SRC + 8       # + 4 query tiles + 4 diag tiles
NPOW = NT_SRC + 4         # chunks carrying power tables (src + query)
NQUAD = NTILES // 4       # 10 quads of 512 rows

_cache = {}


def _build(h: float):
    import concourse.bass as bass
    import concourse.bacc as bacc
    import concourse.tile as tile
    from concourse import mybir

    f32 = mybir.dt.float32
    f32r = mybir.dt.float32r
    bf16 = mybir.dt.bfloat16
    AF = mybir.ActivationFunctionType
    ALU = mybir.AluOpType

    nc = bacc.Bacc("TRN2", target_bir_lowering=False, debug=False, num_devices=1)
    # each 512-col quad is host-stacked to [128, 256] (top=cols 0:256,
    # bottom=cols 256:512) so K=128 without doubling the DMA bytes;
    # w1T carries both weight variants [W1t;0] | [0;W1t]
    xqT = nc.dram_tensor("xqT", [HID, BQ // 2], bf16, kind="ExternalInput").ap()
    tXT = nc.dram_tensor("tXT", [HID, N // 2], bf16, kind="ExternalInput").ap()
    tdT = nc.dram_tensor("tdT", [HID, BQ // 2], bf16, kind="ExternalInput").ap()
    w1T = nc.dram_tensor("w1T", [HID, 2 * HID], bf16, kind="ExternalInput").ap()
    w2T = nc.dram_tensor("w2T", [HID, O], bf16, kind="ExternalInput").ap()
    Yt = nc.dram_tensor("Y", [128, NT_SRC * O], bf16, kind="ExternalInput").ap()
    yd = nc.dram_tensor("yd", [128, 4 * O], bf16, kind="ExternalInput").ap()
    EJ = nc.dram_tensor("EJ", [20, 2 * O * KT], f32, kind="ExternalInput").ap()
    out = nc.dram_tensor("out", [128, 4 * O], f32, kind="ExternalOutput").ap()

    with tile.TileContext(nc) as tc:
        with (
            tc.tile_pool(name="singles", bufs=1) as S,
            tc.tile_pool(name="work", bufs=3) as W,
            tc.tile_pool(name="psW", bufs=1, space="PSUM") as PSW,
            tc.tile_pool(name="psH", bufs=3, space="PSUM") as PSH,
            tc.tile_pool(name="psX", bufs=1, space="PSUM") as PSX,
            tc.tile_pool(name="psM", bufs=1, space="PSUM") as PSM,
        ):
            # ---------------- constants ----------------
            warm = S.tile([1, 16], f32)
            nc.vector.memset(warm, 0.0)
            nc.scalar.activation(out=warm, in_=warm, func=AF.Exp)
            ones128 = S.tile([128, 512], bf16)
            nc.vector.memset(ones128, 1.0)
            ones32f = S.tile([20, 128], f32)
            nc.vector.memset(ones32f, 1.0)
            onesW = S.tile([20, 128], f32r)
            nc.vector.tensor_copy(onesW, ones32f)
            vones = S.tile([128, NPOW * O], f32)
            nc.vector.memset(vones, 1.0)

            # PE HAM warm-up: K=128 bf16 matmuls while inputs stream in
            wps = PSW.tile([128, 512], f32, tag="warm", name="wps")
            for i in range(5):
                nc.tensor.matmul(wps, lhsT=ones128[:, 0:128], rhs=ones128,
                                 start=True, stop=True)

            # -------- input DMAs: weights first, tXT staggered ----------
            # one SBUF tile per DMA: a shared tile would make every MM1
            # conservatively wait on ALL xT writers
            w1sb = S.tile([HID, 2 * HID], bf16)
            nc.sync.dma_start(out=w1sb, in_=w1T)
            w2sb = S.tile([HID, O], bf16)
            nc.sync.dma_start(out=w2sb, in_=w2T)
            cuts = [0, 256, 768, 1280, 2048]     # stacked coords
            xTp = []
            for i in range(4):
                pc = cuts[i + 1] - cuts[i]
                t = S.tile([HID, pc], bf16, name=f"xTp{i}")
                eng = nc.sync if i % 2 == 0 else nc.gpsimd
                eng.dma_start(out=t, in_=tXT[:, cuts[i]:cuts[i + 1]])
                xTp.append(t)
            xTq = S.tile([HID, BQ // 2], bf16, name="xTq")
            nc.sync.dma_start(out=xTq, in_=xqT)
            xTd = S.tile([HID, BQ // 2], bf16, name="xTd")
            nc.gpsimd.dma_start(out=xTd, in_=tdT)
            # quad q -> (piece tile, col offset within piece)
            quad_src = []
            for q in range(8):
                base = q * 256
                i = next(i for i in range(4) if cuts[i] <= base < cuts[i + 1])
                quad_src.append((xTp[i], base - cuts[i]))
            quad_src += [(xTq, 0), (xTd, 0)]
            Ej = S.tile([20, 2 * O * KT], f32)
            nc.scalar.dma_start(out=Ej, in_=EJ)
            Ytab = S.tile([128, NT_SRC * O], bf16)
            nc.scalar.dma_start(out=Ytab, in_=Yt)
            ydT = S.tile([128, 4 * O], bf16)
            nc.scalar.dma_start(out=ydT, in_=yd)

            # ---------------- MM1 + relu -> H ----------------
            H = S.tile([128, NTILES * 128], bf16)
            for q in range(NQUAD):
                src, off = quad_src[q]
                hp = PSH.tile([128, 512], f32, tag="H", name="hps")
                for hh in range(2):
                    nc.tensor.matmul(hp[:, hh * 256:(hh + 1) * 256],
                                     lhsT=w1sb[:, hh * HID:(hh + 1) * HID],
                                     rhs=src[:, off:off + 256],
                                     start=True, stop=True)
                dst = H[:, q * 512:(q + 1) * 512]
                if q % 2 == 0:
                    nc.scalar.activation(out=dst, in_=hp, func=AF.Relu)
                else:
                    nc.vector.tensor_scalar_max(dst, hp, 0.0)

            # ---------------- MM2 -> X' ----------------
            xps = PSX.tile([128, NTILES * O], f32, tag="xp", name="xps")
            for t in range(NTILES):
                nc.tensor.matmul(
                    xps[:, t * O:(t + 1) * O],
                    lhsT=H[:, t * 128:(t + 1) * 128], rhs=w2sb,
                    start=True, stop=True)
            Xp = S.tile([128, NTILES * O], f32)
            nc.scalar.copy(Xp, xps)

            QC = NT_SRC * O          # col offset of query block (320)
            DC = (NT_SRC + 4) * O    # col offset of diag block (360)

            # ---------------- d, c -> cd table ----------------
            sq = S.tile([128, NTILES * O], f32)
            nc.scalar.square(sq, Xp)
            cd = S.tile([128, NT_SRC * 2 * O], f32r)
            cd4 = cd.rearrange("p (c j o) -> p c j o", j=2, o=O)
            nc.scalar.activation(
                out=cd4[:, :, 1, :],
                in_=sq.rearrange("p (c o) -> p c o", o=O)[:, 0:NT_SRC, :],
                func=AF.Exp, scale=-0.5)
            nc.gpsimd.tensor_mul(cd4[:, :, 0, :],
                                 Ytab.rearrange("p (c o) -> p c o", o=O),
                                 cd4[:, :, 1, :])
            # PE keep-alive: the DVE power chain leaves the PE idle >3.4us,
            # which re-throttles HAM; a few K=128 matmuls (one set gated on
            # cd so it fires mid-gap) keep the clock at 2.4 GHz
            onesR = S.tile([128, 128], f32r)
            nc.vector.tensor_copy(onesR, ones128[:, 0:128])
            for i in range(3):
                nc.tensor.matmul(wps[:, 0:128], lhsT=onesR, rhs=onesR,
                                 start=True, stop=True)
            for i in range(3):
                nc.tensor.matmul(wps[:, 0:128], lhsT=onesR, rhs=cd[:, 0:128],
                                 start=True, stop=True)

            # -------- power table: square/multiply ping-pong --------
            # x^2m = Square(x^m) on ACT, x^(2m+1) = x^2m * x on DVE;
            # the serial chain spans two engines instead of one
            V = S.tile([128, NPOW * O * KT], f32r)
            V4 = V.rearrange("p (c o k) -> p c o k", o=O, k=KT)
            Xs4 = Xp.rearrange("p (c o) -> p c o", o=O)[:, 0:NPOW, :]
            nc.vector.tensor_copy(V4[:, :, :, 0],
                                  vones.rearrange("p (c o) -> p c o", o=O))
            nc.scalar.copy(V4[:, :, :, 1], Xs4)
            for k in range(2, KT):
                if k % 2 == 0:
                    nc.scalar.square(V4[:, :, :, k], V4[:, :, :, k // 2])
                else:
                    nc.vector.tensor_mul(V4[:, :, :, k], V4[:, :, :, k - 1],
                                         Xs4)

            # ---------------- moment matmuls ----------------
            mps = PSM.tile([20, O * KT], f32, tag="M", name="mps")
            for c in range(NT_SRC):
                nc.tensor.matmul(
                    mps, lhsT=cd[:, c * 2 * O:(c + 1) * 2 * O],
                    rhs=V[:, c * O * KT:(c + 1) * O * KT],
                    start=(c == 0), stop=(c == NT_SRC - 1))

            # select diag blocks M[j*10+o, (o,k)] (1/k! in the mask) and
            # broadcast to 128 partitions via a ones-weighted matmul
            masked = S.tile([20, 2 * O * KT], f32r)
            nc.vector.tensor_mul(masked[:, 0:O * KT], mps, Ej[:, 0:O * KT])
            nc.vector.tensor_mul(masked[:, O * KT:], mps, Ej[:, O * KT:])
            m2ps = PSX.tile([128, 2 * O * KT], f32, tag="m2", name="m2ps")
            nc.tensor.matmul(m2ps, lhsT=onesW, rhs=masked, start=True, stop=True)
            M2 = S.tile([128, 2 * O * KT], f32)
            nc.scalar.copy(M2, m2ps)

            # ---------------- eval ----------------
            num = S.tile([128, 4 * O], f32)
            den = S.tile([128, 4 * O], f32)
            M2P = M2.ap[0][0]
            UQ = V[:, NT_SRC * O * KT:NPOW * O * KT]   # query powers
            for j, acc in ((0, num), (1, den)):
                m2b = bass.AP(tensor=M2.tensor, offset=M2.offset + j * O * KT,
                              ap=[[M2P, 128], [0, 4], [1, O * KT]])
                p1 = W.tile([128, 4 * O * KT], f32, tag="p1")
                nc.vector.tensor_mul(
                    p1.rearrange("p (qc f) -> p qc f", f=O * KT),
                    UQ.rearrange("p (qc f) -> p qc f", f=O * KT), m2b)
                nc.vector.tensor_reduce(
                    acc, p1.rearrange("p (qc o k) -> p qc o k", o=O, k=KT),
                    axis=mybir.AxisListType.X, op=ALU.add)

            # ---------------- diagonal correction ----------------
            t1 = S.tile([128, 4 * O], f32)
            nc.vector.tensor_mul(t1, Xp[:, DC:DC + 4 * O], Xp[:, QC:QC + 4 * O])
            nc.vector.scalar_tensor_tensor(
                out=t1, in0=sq[:, DC:DC + 4 * O], scalar=-0.5, in1=t1,
                op0=ALU.mult, op1=ALU.add)
            kd = S.tile([128, 4 * O], f32)
            nc.scalar.activation(out=kd, in_=t1, func=AF.Exp)
            nc.vector.tensor_mul(t1, kd, ydT)
            nc.vector.tensor_sub(num, num, t1)
            nc.vector.tensor_sub(den, den, kd)
            rec = S.tile([128, 4 * O], f32)
            nc.vector.reciprocal(rec, den)
            nc.vector.tensor_mul(num, num, rec)

            nc.sync.dma_start(out=out, in_=num)

    nc.compile()
    return nc


def _ej_const():
    """[20, (j,o,k)] mask: row j*10+o keeps block (j, o, :) with value 1/k!."""
    ej = np.zeros((20, 2 * O * KT), np.float32)
    fact = np.cumprod(np.concatenate([[1.0], np.arange(1, KT)])).astype(np.float64)
    for j in range(2):
        for o in range(O):
            ej[j * O + o, (j * O + o) * KT:(j * O + o + 1) * KT] = 1.0 / fact
    return ej


def _stackT(a):
    """[n, 64] f32 -> [128, n/2] bf16: per 512-col quad of a.T, top half
    rows hold cols 0:256, bottom half rows hold cols 256:512 (K=128
    matmuls with the dual zero-padded weight variants, no extra bytes)."""
    import ml_dtypes
    T = a.T.astype(np.float32)                    # [64, n]
    nq = T.shape[1] // 512
    Q = T.reshape(D, nq, 2, 256)
    out = np.concatenate([Q[:, :, 0, :], Q[:, :, 1, :]], axis=0)  # [128,nq,256]
    return np.ascontiguousarray(out.reshape(HID, nq * 256)).astype(ml_dtypes.bfloat16)


def make_in_maps(x, train_X, Y, W1, W2, h):
    import ml_dtypes
    bf = ml_dtypes.bfloat16
    x = np.ascontiguousarray(x, dtype=np.float32)
    train_X = np.ascontiguousarray(train_X, dtype=np.float32)
    Y = np.asarray(Y, np.float32)
    # device layout [128, (t, o)]: partition p holds rows t*128+p
    Ydev = np.ascontiguousarray(
        Y.reshape(NT_SRC, 128, O).transpose(1, 0, 2).reshape(128, NT_SRC * O)
    ).astype(bf)
    tXTp = _stackT(train_X)
    w1f = np.asarray(W1, np.float32).T             # [64, 128]
    zer = np.zeros_like(w1f)
    w1t = np.ascontiguousarray(np.concatenate(
        [np.concatenate([w1f, zer], 0), np.concatenate([zer, w1f], 0)],
        axis=1)).astype(bf)                        # [128, 256] = [w1a | w1b]
    w2t = np.ascontiguousarray(
        (np.asarray(W2, np.float32) / float(h)).T).astype(bf)
    ej = _ej_const()
    in_maps = []
    for c in range(NCORES):
        sl = slice(c * BQ, (c + 1) * BQ)
        yds = np.ascontiguousarray(
            Y[sl].reshape(4, 128, O).transpose(1, 0, 2).reshape(128, 4 * O)
        ).astype(bf)
        in_maps.append({
            "xqT": _stackT(x[sl]),
            "tXT": tXTp,
            "tdT": _stackT(train_X[sl]),
            "w1T": w1t, "w2T": w2t,
            "Y": Ydev, "yd": yds, "EJ": ej,
        })
    return in_maps


def kernel(x, train_X, Y, W1, W2, h):
    import concourse.bass_utils as bass_utils

    hval = float(h)
    key = ("fgt11", hval)
    if key not in _cache:
        _cache[key] = _build(hval)
    nc = _cache[key]

    in_maps = make_in_maps(x, train_X, Y, W1, W2, hval)
    res = bass_utils.run_bass_kernel_spmd(nc, in_maps, core_ids=list(range(NCORES)))
    outs = []
    for c in range(NCORES):
        o = res.results[c]["out"]                      # [128, (qc, o)]
        outs.append(o.reshape(128, 4, O).transpose(1, 0, 2).reshape(BQ, O))
    return np.concatenate(outs, axis=0)
